# revision 23
# baseline (speedup 1.0000x reference)
"""Trainium2 Bass kernel for nn_Decoder_36825049596036.

Decoder step: attention over encoder outputs + single GRU step (zero initial
state) + big vocab projection.

Sharding (8 NeuronCores, one chip):
  - Phase A (attention + GRU): data-parallel over batch, 32 rows/core.
  - AllGather of the GRU state (transposed) across all 8 cores.
  - Phase B (fc, 1024x32000): tensor-parallel over vocab, 4000 cols/core,
    full batch.

All heavy matmuls run in float32r (TF32-like: fp32 storage, ~12-bit mantissa,
1 cycle/row on the PE for moving dims >= 256 vs 4 cycles/row for plain fp32).
Operands are pre-rounded to fp32r on the host so on-device matmuls are exact.

Algebraic simplifications vs the reference:
  - h0 == 0  =>  hk = gru_bias[1]; the recurrent matmul and the z*h term
    vanish; gate biases fold into the input-kernel bias row.
  - softmax is shift-invariant => bv drops out of attn weights entirely.
  - b1 + b2 fold into one bias row added via an expander matmul that also
    broadcasts (hidden @ W2) over the seq axis inside PSUM accumulation.
"""

import numpy as np

import concourse.bass as bass
import concourse.mybir as mybir
import concourse.tile as tile
from concourse import bacc
from concourse.bass import ds, ts
from concourse.bass_utils import run_bass_kernel_spmd
from concourse.masks import make_identity
from neuron_dtypes import static_cast_fp32_to_fp32r

B, S, U, E, V_SZ = 256, 64, 1024, 256, 32000
NCORES = 8
BL = B // NCORES          # 32 local batch rows
VL = V_SZ // NCORES       # 4000 local vocab cols
RSEQ = BL * S             # 2048 local (batch, seq) rows
P = 128
KU = U // P               # 8 u-chunks
KI = (U + E) // P         # 10 input chunks for the GRU kernel
VBLK = 500                # fc vocab block
NVB = VL // VBLK          # 8 blocks
F32 = mybir.dt.float32
F16 = mybir.dt.float16

LAST_EXEC_NS = None       # filled when kernel() runs with profiling on
PROFILE = False

_CACHE = {}


def _round_r(x):
    """Round an fp32 array to fp32r bit patterns (returned as fp32)."""
    x = np.ascontiguousarray(x, dtype=np.float32)
    return np.ascontiguousarray(
        np.asarray(static_cast_fp32_to_fp32r(x)).view(np.float32).reshape(x.shape)
    )


def _build():
    nc = bacc.Bacc("TRN2", target_bir_lowering=False, debug=False,
                   num_devices=NCORES)

    def din(name, shape, dtype=F16):
        return nc.dram_tensor(name, list(shape), dtype,
                              kind="ExternalInput").ap()

    encT_d = din("encT", (U, RSEQ))               # host-transposed
    enc_d = din("enc", (RSEQ, U))                 # natural, rows = b*64+s
    hT_d = din("hiddenT", (U, BL))
    xeT_d = din("xeT", (E, BL), F16)
    w1_d = din("w1", (U, U))
    w2_d = din("w2", (U, U))
    b12_d = din("b12", (1, U))
    v2_d = din("v2", (U, 2))
    gruk_d = din("gruk", (U + E, 3 * U), F16)
    gb3_d = din("gbias3", (1, 3 * U), F16)
    gb1h_d = din("gb1h", (BL, U), F32)            # replicated rows, plain f32
    exp_d = din("expander", (BL + 1, RSEQ))
    ones_d = din("ones1", (1, P))
    onesb_d = din("ones_bf", (1, P), F16)
    zblk_d = din("zeros_blk", (P, 16 * BL))
    fcw_d = din("fcw", (U, VL), F16)
    fcb_d = din("fcb", (1, VL), F16)

    out_d = nc.dram_tensor("out", [B, VL], F32, kind="ExternalOutput").ap()
    st_d = nc.dram_tensor("state_out", [BL, U], F32, kind="ExternalOutput").ap()
    aw_d = nc.dram_tensor("attnw_out", [BL, S], F16, kind="ExternalOutput").ap()

    with tile.TileContext(nc) as tc:
        with (
            tc.tile_pool(name="const", bufs=1) as cpool,
            tc.tile_pool(name="dram", bufs=1, space="DRAM") as dram,
        ):
            ident = cpool.tile([P, P], F32)
            make_identity(nc, ident)
            identh = cpool.tile([BL, BL], F16)
            make_identity(nc, identh)
            w_big = cpool.tile([P, 16, BL], F16)
            nc.sync.dma_start(out=w_big.rearrange("p a b -> p (a b)"), in_=zblk_d)
            ones_sb = cpool.tile([1, P], F16)
            nc.sync.dma_start(out=ones_sb[:], in_=ones_d)
            onesb_sb = cpool.tile([1, P], F16)
            nc.sync.dma_start(out=onesb_sb[:], in_=onesb_d)
            v2_sb = cpool.tile([P, KU, 2], F16)
            nc.sync.dma_start(out=v2_sb[:], in_=v2_d.rearrange("(k p) c -> p k c", p=P))
            hT_sb = cpool.tile([P, KU, BL], F16)
            nc.sync.dma_start(out=hT_sb[:], in_=hT_d.rearrange("(k p) b -> p k b", p=P))
            exp_sb = cpool.tile([BL + 1, RSEQ], F16)
            nc.sync.dma_start(out=exp_sb[:], in_=exp_d)
            hw2x = cpool.tile([BL + 1, U], F16)
            nc.sync.dma_start(out=hw2x[BL:BL + 1, :], in_=b12_d)
            logits_sb = cpool.tile([1, RSEQ], F32)
            inpT = cpool.tile([P, KI, BL], F16)
            nc.sync.dma_start(out=inpT[:, KU:KI, :],
                              in_=xeT_d.rearrange("(j p) b -> p j b", p=P))

            ag_in = dram.tile([U, BL], F16)
            ag_out = dram.tile([NCORES * U, BL], F16, addr_space="Shared")

            # encT: the A1 moving operand, fully resident (64 KB/part)
            encT_cm = tc.tile_pool(name="encTp", bufs=1)
            encTpool = encT_cm.__enter__()
            encT = encTpool.tile([P, KU, RSEQ], F16)
            nc.sync.dma_start(out=encT[:],
                              in_=encT_d.rearrange("(k p) r -> p k r", p=P))

            # ---- hidden @ W2 (+bias via row BL of hw2x) -------------------
            with (
                tc.tile_pool(name="w2", bufs=1) as w2pool,
                tc.tile_pool(name="psh2", bufs=2, space="PSUM") as psh2,
            ):
                w2_sb = w2pool.tile([P, KU, U], F16)
                nc.sync.dma_start(out=w2_sb[:],
                                  in_=w2_d.rearrange("(k p) n -> p k n", p=P))
                for nn in range(2):
                    ph = psh2.tile([BL, 512], F32)
                    for k in range(KU):
                        nc.tensor.matmul(ph[:], hT_sb[:, k, :],
                                         w2_sb[:, k, ts(nn, 512)],
                                         start=(k == 0), stop=(k == KU - 1))
                    nc.scalar.copy(hw2x[0:BL, ts(nn, 512)], ph[:])

            # prefetch pools on the right-side stack. DMA *emission order*
            # matters: each HW queue drains in order, so a backpressured DMA
            # blocks everything emitted after it on the same queue.  Emit the
            # immediately-runnable prefetches (slots free) first, then the
            # backpressured remainder in consumption order (A1 -> ctx -> GRU
            # -> fc).
            gk_cm = tc.tile_pool(name="gruk", bufs=4, side="right")
            gkpool = gk_cm.__enter__()
            gruk_r = gruk_d.rearrange("(k p) n -> k p n", p=P)
            gk_tiles = [gkpool.tile([P, 3 * U], F16, name=f"gk{k}", tag="gk")
                        for k in range(KI)]

            enc2_cm = tc.tile_pool(name="enc2", bufs=4, side="right")
            enc2pool = enc2_cm.__enter__()
            enc2_tiles = [enc2pool.tile([P, U], F16, name=f"e2_{t}", tag="e2")
                          for t in range(16)]

            W1M_BUFS = 3
            w1_cm = tc.tile_pool(name="w1m", bufs=W1M_BUFS)
            w1pool = w1_cm.__enter__()
            w1_r = w1_d.rearrange("(k p) m -> p k m", p=P)
            w1m_tiles = [w1pool.tile([P, KU, P], F16, name=f"w1m{m}", tag="w1m")
                         for m in range(KU)]

            # pass 1: slots free right now
            for m in range(W1M_BUFS):
                nc.sync.dma_start(out=w1m_tiles[m][:], in_=w1_r[:, :, ts(m, P)])
            for t in range(4):
                nc.sync.dma_start(out=enc2_tiles[t][:], in_=enc_d[ts(t, P), :])
            for k in range(4):
                nc.sync.dma_start(out=gk_tiles[k][:], in_=gruk_r[k])
            # pass 2: backpressured, in consumption order
            for m in range(W1M_BUFS, KU):
                nc.sync.dma_start(out=w1m_tiles[m][:], in_=w1_r[:, :, ts(m, P)])
            for t in range(4, 16):
                nc.sync.dma_start(out=enc2_tiles[t][:], in_=enc_d[ts(t, P), :])
            for k in range(4, KI):
                nc.sync.dma_start(out=gk_tiles[k][:], in_=gruk_r[k])

            # ---- A1: score^T = tanh(W1^T enc^T + bcast(hw2) + b12), logits
            with (
                tc.tile_pool(name="pss", bufs=3, space="PSUM") as pss,
                tc.tile_pool(name="psl", bufs=1, space="PSUM") as psl,
                tc.tile_pool(name="score", bufs=8) as scpool,
            ):
                pl = [psl.tile([2, 512], F32, name=f"pl{i}", tag=f"pl{i}")
                      for i in range(4)]
                sc_prev = None
                for m in range(KU):
                    w1m = w1m_tiles[m]
                    sc_cur = []
                    for n in range(4):
                        ps = pss.tile([P, 512], F32)
                        for k in range(KU):
                            nc.tensor.matmul(ps[:], w1m[:, k, :],
                                             encT[:, k, ts(n, 512)],
                                             start=(k == 0), stop=False)
                        nc.tensor.matmul(ps[:], hw2x[:, ts(m, P)],
                                         exp_sb[:, ts(n, 512)],
                                         start=False, stop=True)
                        sc = scpool.tile([P, 512], F16)
                        nc.scalar.activation(sc[:], ps[:],
                                             mybir.ActivationFunctionType.Tanh)
                        sc_cur.append(sc)
                    # logits matmuls lag one m-round so the in-order PE queue
                    # never waits on the tanh that produces their input
                    if sc_prev is not None:
                        for n in range(4):
                            nc.tensor.matmul(pl[n][:], v2_sb[:, m - 1, :],
                                             sc_prev[n][:],
                                             start=(m - 1 == 0), stop=False,
                                             skip_group_check=True)
                    sc_prev = sc_cur
                for n in range(4):
                    nc.tensor.matmul(pl[n][:], v2_sb[:, KU - 1, :],
                                     sc_prev[n][:],
                                     start=False, stop=True,
                                     skip_group_check=True)
                for n in range(4):
                    nc.scalar.copy(logits_sb[0:1, ts(n, 512)], pl[n][0:1, :])
            w1_cm.__exit__(None, None, None)
            encT_cm.__exit__(None, None, None)

            # fcw prefetch: opened as soon as encT's space frees; all eight
            # blocks resident before fc needs them
            fcw_cm = tc.tile_pool(name="fcw", bufs=1)
            fcwpool = fcw_cm.__enter__()
            fcw_r = fcw_d.rearrange("(k p) (v n) -> v p k n", p=P, n=VBLK)
            fw_tiles = [fcwpool.tile([P, KU, VBLK], F16,
                                     name=f"fw{v}", tag=f"fw{v}")
                        for v in range(NVB)]
            for v in range(NVB):
                nc.sync.dma_start(out=fw_tiles[v][:], in_=fcw_r[v])

            # ---- softmax over seq (relayout to [32b, 64s]) ----------------
            aw_raw = cpool.tile([BL, S], F32)
            nc.sync.dma_start(out=aw_raw[:], in_=logits_sb[0:1, :])
            negmx = cpool.tile([BL, 1], F32)
            nc.vector.tensor_reduce(negmx[:], aw_raw[:],
                                    axis=mybir.AxisListType.X,
                                    op=mybir.AluOpType.max, negate=True)
            ew = cpool.tile([BL, S], F32)
            sm = cpool.tile([BL, 1], F32)
            nc.scalar.activation(ew[:], aw_raw[:],
                                 mybir.ActivationFunctionType.Exp,
                                 bias=negmx[:], accum_out=sm[:])
            rs = cpool.tile([BL, 1], F32)
            nc.vector.reciprocal(rs[:], sm[:])
            awf = cpool.tile([BL, S], F16)
            nc.vector.tensor_scalar_mul(awf[:], ew[:], rs[:])
            nc.sync.dma_start(out=aw_d, in_=awf[:])

            # ---- context (natural layout) via per-pair block weights ------
            with tc.tile_pool(name="psctx", bufs=1, space="PSUM") as psctx:
                paw = psctx.tile([S, BL], F16)
                nc.tensor.transpose(paw[:], awf[:], identh[:])
                awt = cpool.tile([S, BL], F16)
                nc.vector.tensor_copy(awt[:], paw[:])
                wbf = w_big.rearrange("p a b -> p (a b)")
                nc.sync.dma_start(out=wbf[0:S, 0:512:34], in_=awt[:, 0:BL:2])
                nc.sync.dma_start(out=wbf[S:P, 1:512:34], in_=awt[:, 1:BL:2])

                # ctx_nat[b, u] accumulated over the 16 batch pairs: w_big[t]
                # is zero outside pair t's columns, so stray products vanish
                pcn = psctx.tile([BL, U], F32)
                for t in range(16):
                    for h in range(2):
                        nc.tensor.matmul(
                            pcn[:, ts(h, 512)],
                            w_big[:, t, :],
                            enc2_tiles[t][:, ts(h, 512)],
                            start=(t == 0), stop=(t == 15),
                            skip_group_check=True)
                ctx_nat = cpool.tile([BL, U], F16)
                nc.scalar.copy(ctx_nat[:], pcn[:])
                with tc.tile_pool(name="psct2", bufs=2, space="PSUM") as psct2:
                    for j in range(KU):
                        ptj = psct2.tile([P, BL], F16)
                        nc.tensor.transpose(ptj[:], ctx_nat[:, ts(j, P)],
                                            identh[:])
                        nc.vector.tensor_copy(inpT[:, j, :], ptj[:])
            enc2_cm.__exit__(None, None, None)

            # ---- GRU: xk = inp @ gruk + bias row, then gates --------------
            with (
                tc.tile_pool(name="psxk", bufs=1, space="PSUM") as psxk,
                tc.tile_pool(name="gwork", bufs=1) as gwork,
                tc.tile_pool(name="psst", bufs=2, space="PSUM") as psst,
                tc.tile_pool(name="stt", bufs=2) as sttpool,
            ):
                gb3_sb = gwork.tile([1, 3 * U], F16)
                nc.sync.dma_start(out=gb3_sb[:], in_=gb3_d)
                gb1h_sb = gwork.tile([BL, U], F32)
                nc.sync.dma_start(out=gb1h_sb[:], in_=gb1h_d)

                pxk = [psxk.tile([BL, 512], F32, name=f"pxk{i}", tag=f"pxk{i}")
                       for i in range(6)]
                for k in range(KI):
                    for nk in range(6):
                        nc.tensor.matmul(pxk[nk][:], inpT[:, k, :],
                                         gk_tiles[k][:, ts(nk, 512)],
                                         start=(k == 0), stop=False,
                                         skip_group_check=True)
                for nk in range(6):
                    nc.tensor.matmul(pxk[nk][:], onesb_sb[0:1, 0:BL],
                                     gb3_sb[0:1, ts(nk, 512)],
                                     start=False, stop=True,
                                     skip_group_check=True)
                gk_cm.__exit__(None, None, None)

                z_sb = gwork.tile([BL, U], F32)
                r_sb = gwork.tile([BL, U], F32)
                for h in range(2):
                    nc.scalar.activation(z_sb[:, ts(h, 512)], pxk[h][:],
                                         mybir.ActivationFunctionType.Sigmoid)
                    nc.scalar.activation(r_sb[:, ts(h, 512)], pxk[2 + h][:],
                                         mybir.ActivationFunctionType.Sigmoid)
                t1 = gwork.tile([BL, U], F32)
                nc.vector.tensor_mul(t1[:], r_sb[:], gb1h_sb[:])
                hpre = gwork.tile([BL, U], F32)
                for h in range(2):
                    nc.vector.tensor_add(hpre[:, ts(h, 512)],
                                         t1[:, ts(h, 512)], pxk[4 + h][:])
                hh = gwork.tile([BL, U], F32)
                nc.scalar.activation(hh[:], hpre[:],
                                     mybir.ActivationFunctionType.Tanh)
                t3 = gwork.tile([BL, U], F32)
                nc.vector.tensor_mul(t3[:], z_sb[:], hh[:])
                state_sb = gwork.tile([BL, U], F32)
                nc.vector.tensor_sub(state_sb[:], hh[:], t3[:])
                nc.sync.dma_start(out=st_d, in_=state_sb[:])

                # state^T -> DRAM staging -> AllGather
                for j in range(KU):
                    pst = psst.tile([P, BL], F32)
                    nc.tensor.transpose(pst[:], state_sb[:, ts(j, P)],
                                        ident[0:BL, 0:BL])
                    stt = sttpool.tile([P, BL], F16)
                    nc.vector.tensor_copy(stt[:], pst[:])
                    nc.sync.dma_start(out=ag_in[ts(j, P), :], in_=stt[:])
                nc.gpsimd.collective_compute(
                    "AllGather", mybir.AluOpType.bypass,
                    replica_groups=[list(range(NCORES))],
                    ins=[ag_in.opt()], outs=[ag_out.opt()])

            # ---- fc: out = state_full @ fcw_shard + fcb -------------------
            with (
                tc.tile_pool(name="stf", bufs=1) as stfpool,
                tc.tile_pool(name="psfc", bufs=3, space="PSUM") as psfc,
                tc.tile_pool(name="osb", bufs=3) as opool,
                tc.tile_pool(name="fcb", bufs=1) as fcbpool,
            ):
                fcb_sb = fcbpool.tile([1, VL], F16)
                nc.sync.dma_start(out=fcb_sb[:], in_=fcb_d)
                stf = stfpool.tile([P, KU, NCORES, BL], F16)
                ag_r = ag_out.rearrange("(c k p) b -> k p c b", c=NCORES, p=P)
                for k in range(KU):
                    nc.sync.dma_start(out=stf[:, k], in_=ag_r[k])
                for v in range(NVB):
                    for mc in range(2):
                        pfc = psfc.tile([P, VBLK], F32)
                        for kc in range(KU):
                            nc.tensor.matmul(
                                pfc[:],
                                stf[:, kc, ds(4 * mc, 4), :],
                                fw_tiles[v][:, kc, :],
                                start=(kc == 0), stop=False,
                                skip_group_check=True)
                        nc.tensor.matmul(pfc[:], onesb_sb[0:1, :],
                                         fcb_sb[0:1, ds(v * VBLK, VBLK)],
                                         start=False, stop=True,
                                         skip_group_check=True)
                        ou = opool.tile([P, VBLK], F32)
                        nc.scalar.copy(ou[:], pfc[:])
                        nc.sync.dma_start(out=out_d[ts(mc, P), ds(v * VBLK, VBLK)],
                                          in_=ou[:])
            fcw_cm.__exit__(None, None, None)

    nc.compile()
    return nc


def _host_prep(x, hidden, encode_output, embedding, W1, b1, W2, b2, V, bv,
               gru_kernel, gru_rec_kernel, gru_bias, fc_W, fc_b):
    f32, f16 = np.float32, np.float16
    x_idx = np.asarray(x).reshape(-1).astype(np.int64)
    xe = np.asarray(embedding, dtype=f32)[x_idx]          # (B, E)
    W1h = np.asarray(W1, f16)
    W2h = np.asarray(W2, f16)
    b12 = (np.asarray(b1, f32) + np.asarray(b2, f32)).reshape(1, U).astype(f16)
    v2 = np.zeros((U, 2), f16)
    v2[:, 0] = np.asarray(V, f32).reshape(-1).astype(f16)
    grukh = np.asarray(gru_kernel, f16)
    gb = np.asarray(gru_bias, f32)
    gb3 = np.concatenate([gb[0, :2 * U] + gb[1, :2 * U], gb[0, 2 * U:]])
    gb3 = gb3.reshape(1, 3 * U).astype(f16)
    gb1h = np.ascontiguousarray(
        np.broadcast_to(gb[1, 2 * U:].astype(f32), (BL, U)))
    expander = np.zeros((BL + 1, RSEQ), f16)
    for b in range(BL):
        expander[b, b * S:(b + 1) * S] = 1.0
    expander[BL, :] = 1.0
    ones1 = np.ones((1, P), f16)
    ones_bf = np.ones((1, P), f16)
    zeros_blk = np.zeros((P, 16 * BL), f16)
    fcwh = np.asarray(fc_W, f16)
    fcbh = np.asarray(fc_b, f32).reshape(1, V_SZ).astype(f16)
    ench = np.asarray(encode_output, f16)
    hidh = np.asarray(hidden, f16)
    xeh = np.asarray(xe, f16)

    in_maps = []
    for c in range(NCORES):
        sl = slice(c * BL, (c + 1) * BL)
        enc_c = np.ascontiguousarray(ench[sl].reshape(RSEQ, U))
        in_maps.append({
            "enc": enc_c,
            "encT": np.ascontiguousarray(enc_c.T),
            "hiddenT": np.ascontiguousarray(hidh[sl].T),
            "xeT": np.ascontiguousarray(xeh[sl].T),
            "w1": W1h, "w2": W2h, "b12": b12, "v2": v2,
            "gruk": grukh, "gbias3": gb3, "gb1h": gb1h,
            "expander": expander, "ones1": ones1, "ones_bf": ones_bf,
            "zeros_blk": zeros_blk,
            "fcw": np.ascontiguousarray(fcwh[:, c * VL:(c + 1) * VL]),
            "fcb": np.ascontiguousarray(fcbh[:, c * VL:(c + 1) * VL]),
        })
    return in_maps


def kernel(**inputs):
    global LAST_EXEC_NS
    if "nc" not in _CACHE:
        _CACHE["nc"] = _build()
    nc = _CACHE["nc"]
    in_maps = _host_prep(**inputs)
    res = run_bass_kernel_spmd(nc, in_maps, core_ids=list(range(NCORES)),
                               trace=PROFILE)
    LAST_EXEC_NS = res.exec_time_ns
    out = np.concatenate([res.results[c]["out"] for c in range(NCORES)], axis=1)
    state = np.concatenate([res.results[c]["state_out"] for c in range(NCORES)],
                           axis=0)
    attn_w = np.concatenate([res.results[c]["attnw_out"] for c in range(NCORES)],
                            axis=0)[:, :, None]
    return out.astype(np.float32), state.astype(np.float32), attn_w.astype(np.float32)


# revision 26
# speedup vs baseline: 1.0337x; 1.0337x over previous
"""Trainium2 Bass kernel for nn_Decoder_36825049596036.

Decoder step: attention over encoder outputs + single GRU step (zero initial
state) + big vocab projection.

Sharding (8 NeuronCores, one chip):
  - Phase A (attention + GRU): data-parallel over batch, 32 rows/core.
  - AllGather of the GRU state (transposed) across all 8 cores.
  - Phase B (fc, 1024x32000): tensor-parallel over vocab, 4000 cols/core,
    full batch.

All heavy matmuls run in float32r (TF32-like: fp32 storage, ~12-bit mantissa,
1 cycle/row on the PE for moving dims >= 256 vs 4 cycles/row for plain fp32).
Operands are pre-rounded to fp32r on the host so on-device matmuls are exact.

Algebraic simplifications vs the reference:
  - h0 == 0  =>  hk = gru_bias[1]; the recurrent matmul and the z*h term
    vanish; gate biases fold into the input-kernel bias row.
  - softmax is shift-invariant => bv drops out of attn weights entirely.
  - b1 + b2 fold into one bias row added via an expander matmul that also
    broadcasts (hidden @ W2) over the seq axis inside PSUM accumulation.
"""

import numpy as np

import concourse.bass as bass
import concourse.mybir as mybir
import concourse.tile as tile
from concourse import bacc
from concourse.bass import ds, ts
from concourse.bass_utils import run_bass_kernel_spmd
from concourse.masks import make_identity
from neuron_dtypes import static_cast_fp32_to_fp32r

B, S, U, E, V_SZ = 256, 64, 1024, 256, 32000
NCORES = 8
BL = B // NCORES          # 32 local batch rows
VL = V_SZ // NCORES       # 4000 local vocab cols
RSEQ = BL * S             # 2048 local (batch, seq) rows
P = 128
KU = U // P               # 8 u-chunks
KI = (U + E) // P         # 10 input chunks for the GRU kernel
VBLK = 500                # fc vocab block
NVB = VL // VBLK          # 8 blocks
F32 = mybir.dt.float32
F16 = mybir.dt.float16

LAST_EXEC_NS = None       # filled when kernel() runs with profiling on
PROFILE = False

_CACHE = {}


def _round_r(x):
    """Round an fp32 array to fp32r bit patterns (returned as fp32)."""
    x = np.ascontiguousarray(x, dtype=np.float32)
    return np.ascontiguousarray(
        np.asarray(static_cast_fp32_to_fp32r(x)).view(np.float32).reshape(x.shape)
    )


def _build():
    nc = bacc.Bacc("TRN2", target_bir_lowering=False, debug=False,
                   num_devices=NCORES)

    def din(name, shape, dtype=F16):
        return nc.dram_tensor(name, list(shape), dtype,
                              kind="ExternalInput").ap()

    encT_d = din("encT", (U, RSEQ))               # host-transposed
    enc_d = din("enc", (RSEQ, U))                 # natural, rows = b*64+s
    hT_d = din("hiddenT", (U, BL))
    xeT_d = din("xeT", (E, BL), F16)
    w1_d = din("w1", (U, U))
    w2_d = din("w2", (U, U))
    b12_d = din("b12", (1, U))
    v2_d = din("v2", (U, 2))
    gruk_d = din("gruk", (U + E, 3 * U), F16)
    gb3_d = din("gbias3", (1, 3 * U), F16)
    gb1h_d = din("gb1h", (BL, U), F32)            # replicated rows, plain f32
    exp_d = din("expander", (BL + 1, RSEQ))
    ones_d = din("ones1", (1, P))
    onesb_d = din("ones_bf", (1, P), F16)
    zblk_d = din("zeros_blk", (P, 16 * BL))
    fcw_d = din("fcw", (U, VL), F16)
    fcb_d = din("fcb", (1, VL), F16)

    out_d = nc.dram_tensor("out", [B, VL], F32, kind="ExternalOutput").ap()
    st_d = nc.dram_tensor("state_out", [BL, U], F32, kind="ExternalOutput").ap()
    aw_d = nc.dram_tensor("attnw_out", [BL, S], F16, kind="ExternalOutput").ap()

    with tile.TileContext(nc) as tc:
        with (
            tc.tile_pool(name="const", bufs=1) as cpool,
            tc.tile_pool(name="dram", bufs=1, space="DRAM") as dram,
        ):
            ident = cpool.tile([P, P], F32)
            make_identity(nc, ident)
            identh = cpool.tile([BL, BL], F16)
            make_identity(nc, identh)
            w_big = cpool.tile([P, 16, BL], F16)
            nc.sync.dma_start(out=w_big.rearrange("p a b -> p (a b)"), in_=zblk_d)
            ones_sb = cpool.tile([1, P], F16)
            nc.sync.dma_start(out=ones_sb[:], in_=ones_d)
            onesb_sb = cpool.tile([1, P], F16)
            nc.sync.dma_start(out=onesb_sb[:], in_=onesb_d)
            v2_sb = cpool.tile([P, KU, 2], F16)
            nc.sync.dma_start(out=v2_sb[:], in_=v2_d.rearrange("(k p) c -> p k c", p=P))
            hT_sb = cpool.tile([P, KU, BL], F16)
            nc.sync.dma_start(out=hT_sb[:], in_=hT_d.rearrange("(k p) b -> p k b", p=P))
            exp_sb = cpool.tile([BL + 1, RSEQ], F16)
            nc.sync.dma_start(out=exp_sb[:], in_=exp_d)
            hw2x = cpool.tile([BL + 1, U], F16)
            nc.sync.dma_start(out=hw2x[BL:BL + 1, :], in_=b12_d)
            logits_sb = cpool.tile([1, RSEQ], F32)
            inpT = cpool.tile([P, KI, BL], F16)
            nc.sync.dma_start(out=inpT[:, KU:KI, :],
                              in_=xeT_d.rearrange("(j p) b -> p j b", p=P))

            ag_in = dram.tile([U, BL], F16)
            ag_out = dram.tile([NCORES * U, BL], F16, addr_space="Shared")

            # encT: the A1 moving operand, fully resident (64 KB/part)
            encT_cm = tc.tile_pool(name="encTp", bufs=1)
            encTpool = encT_cm.__enter__()
            encT = encTpool.tile([P, KU, RSEQ], F16)
            nc.sync.dma_start(out=encT[:],
                              in_=encT_d.rearrange("(k p) r -> p k r", p=P))

            # ---- hidden @ W2 (+bias via row BL of hw2x) -------------------
            with (
                tc.tile_pool(name="w2", bufs=1) as w2pool,
                tc.tile_pool(name="psh2", bufs=2, space="PSUM") as psh2,
            ):
                w2_sb = w2pool.tile([P, KU, U], F16)
                nc.sync.dma_start(out=w2_sb[:],
                                  in_=w2_d.rearrange("(k p) n -> p k n", p=P))
                for nn in range(2):
                    ph = psh2.tile([BL, 512], F32)
                    for k in range(KU):
                        nc.tensor.matmul(ph[:], hT_sb[:, k, :],
                                         w2_sb[:, k, ts(nn, 512)],
                                         start=(k == 0), stop=(k == KU - 1))
                    nc.scalar.copy(hw2x[0:BL, ts(nn, 512)], ph[:])

            # prefetch pools on the right-side stack. DMA *emission order*
            # matters: each HW queue drains in order, so a backpressured DMA
            # blocks everything emitted after it on the same queue.  Emit the
            # immediately-runnable prefetches (slots free) first, then the
            # backpressured remainder in consumption order (A1 -> ctx -> GRU
            # -> fc).
            gk_cm = tc.tile_pool(name="gruk", bufs=4, side="right")
            gkpool = gk_cm.__enter__()
            gruk_r = gruk_d.rearrange("(k p) n -> k p n", p=P)
            gk_tiles = [gkpool.tile([P, 3 * U], F16, name=f"gk{k}", tag="gk")
                        for k in range(KI)]

            enc2_cm = tc.tile_pool(name="enc2", bufs=4, side="right")
            enc2pool = enc2_cm.__enter__()
            enc2_tiles = [enc2pool.tile([P, U], F16, name=f"e2_{t}", tag="e2")
                          for t in range(16)]

            W1M_BUFS = 3
            w1_cm = tc.tile_pool(name="w1m", bufs=W1M_BUFS)
            w1pool = w1_cm.__enter__()
            w1_r = w1_d.rearrange("(k p) m -> p k m", p=P)
            w1m_tiles = [w1pool.tile([P, KU, P], F16, name=f"w1m{m}", tag="w1m")
                         for m in range(KU)]

            # pass 1: slots free right now
            for m in range(W1M_BUFS):
                nc.sync.dma_start(out=w1m_tiles[m][:], in_=w1_r[:, :, ts(m, P)])
            for t in range(4):
                nc.sync.dma_start(out=enc2_tiles[t][:], in_=enc_d[ts(t, P), :])
            for k in range(4):
                nc.sync.dma_start(out=gk_tiles[k][:], in_=gruk_r[k])
            # pass 2: backpressured, in consumption order
            for m in range(W1M_BUFS, KU):
                nc.sync.dma_start(out=w1m_tiles[m][:], in_=w1_r[:, :, ts(m, P)])
            for t in range(4, 16):
                nc.sync.dma_start(out=enc2_tiles[t][:], in_=enc_d[ts(t, P), :])
            for k in range(4, KI):
                nc.sync.dma_start(out=gk_tiles[k][:], in_=gruk_r[k])

            # ---- A1: score^T = tanh(W1^T enc^T + bcast(hw2) + b12), logits
            with (
                tc.tile_pool(name="pss", bufs=3, space="PSUM") as pss,
                tc.tile_pool(name="psl", bufs=1, space="PSUM") as psl,
                tc.tile_pool(name="score", bufs=8) as scpool,
            ):
                pl = [psl.tile([2, 512], F32, name=f"pl{i}", tag=f"pl{i}")
                      for i in range(4)]
                sc_prev = None
                for m in range(KU):
                    w1m = w1m_tiles[m]
                    sc_cur = []
                    for n in range(4):
                        ps = pss.tile([P, 512], F32)
                        for k in range(KU):
                            nc.tensor.matmul(ps[:], w1m[:, k, :],
                                             encT[:, k, ts(n, 512)],
                                             start=(k == 0), stop=False)
                        nc.tensor.matmul(ps[:], hw2x[:, ts(m, P)],
                                         exp_sb[:, ts(n, 512)],
                                         start=False, stop=True)
                        sc = scpool.tile([P, 512], F16)
                        nc.scalar.activation(sc[:], ps[:],
                                             mybir.ActivationFunctionType.Tanh)
                        sc_cur.append(sc)
                    # logits matmuls lag one m-round so the in-order PE queue
                    # never waits on the tanh that produces their input
                    if sc_prev is not None:
                        for n in range(4):
                            nc.tensor.matmul(pl[n][:], v2_sb[:, m - 1, :],
                                             sc_prev[n][:],
                                             start=(m - 1 == 0), stop=False,
                                             skip_group_check=True)
                    sc_prev = sc_cur
                for n in range(4):
                    nc.tensor.matmul(pl[n][:], v2_sb[:, KU - 1, :],
                                     sc_prev[n][:],
                                     start=False, stop=True,
                                     skip_group_check=True)
                for n in range(4):
                    nc.scalar.copy(logits_sb[0:1, ts(n, 512)], pl[n][0:1, :])
            w1_cm.__exit__(None, None, None)
            encT_cm.__exit__(None, None, None)

            # fcw prefetch: opened as soon as encT's space frees; all eight
            # blocks resident before fc needs them
            fcw_cm = tc.tile_pool(name="fcw", bufs=1)
            fcwpool = fcw_cm.__enter__()
            fcw_r = fcw_d.rearrange("(k p) (v n) -> v p k n", p=P, n=VBLK)
            fw_tiles = [fcwpool.tile([P, KU, VBLK], F16,
                                     name=f"fw{v}", tag=f"fw{v}")
                        for v in range(NVB)]
            for v in range(NVB):
                nc.sync.dma_start(out=fw_tiles[v][:], in_=fcw_r[v])

            # ---- softmax over seq (relayout to [32b, 64s]) ----------------
            aw_raw = cpool.tile([BL, S], F32)
            nc.sync.dma_start(out=aw_raw[:], in_=logits_sb[0:1, :])
            negmx = cpool.tile([BL, 1], F32)
            nc.vector.tensor_reduce(negmx[:], aw_raw[:],
                                    axis=mybir.AxisListType.X,
                                    op=mybir.AluOpType.max, negate=True)
            ew = cpool.tile([BL, S], F32)
            sm = cpool.tile([BL, 1], F32)
            nc.scalar.activation(ew[:], aw_raw[:],
                                 mybir.ActivationFunctionType.Exp,
                                 bias=negmx[:], accum_out=sm[:])
            rs = cpool.tile([BL, 1], F32)
            nc.vector.reciprocal(rs[:], sm[:])
            awf = cpool.tile([BL, S], F16)
            nc.vector.tensor_scalar_mul(awf[:], ew[:], rs[:])
            nc.sync.dma_start(out=aw_d, in_=awf[:])

            # ---- context (natural layout) via per-pair block weights ------
            with tc.tile_pool(name="psctx", bufs=1, space="PSUM") as psctx:
                paw = psctx.tile([S, BL], F16)
                nc.tensor.transpose(paw[:], awf[:], identh[:])
                awt = cpool.tile([S, BL], F16)
                nc.vector.tensor_copy(awt[:], paw[:])
                wbf = w_big.rearrange("p a b -> p (a b)")
                nc.sync.dma_start(out=wbf[0:S, 0:512:34], in_=awt[:, 0:BL:2])
                nc.sync.dma_start(out=wbf[S:P, 1:512:34], in_=awt[:, 1:BL:2])

                # ctx_nat[b, u] accumulated over the 16 batch pairs: w_big[t]
                # is zero outside pair t's columns, so stray products vanish
                pcn = psctx.tile([BL, U], F32)
                for t in range(16):
                    for h in range(2):
                        nc.tensor.matmul(
                            pcn[:, ts(h, 512)],
                            w_big[:, t, :],
                            enc2_tiles[t][:, ts(h, 512)],
                            start=(t == 0), stop=(t == 15),
                            skip_group_check=True)
                ctx_nat = cpool.tile([BL, U], F16)
                nc.scalar.copy(ctx_nat[:], pcn[:])
                with tc.tile_pool(name="psct2", bufs=2, space="PSUM") as psct2:
                    for j in range(KU):
                        ptj = psct2.tile([P, BL], F16)
                        nc.tensor.transpose(ptj[:], ctx_nat[:, ts(j, P)],
                                            identh[:])
                        nc.vector.tensor_copy(inpT[:, j, :], ptj[:])
            enc2_cm.__exit__(None, None, None)

            # ---- GRU: xk = inp @ gruk + bias row, then gates --------------
            with (
                tc.tile_pool(name="psxk", bufs=1, space="PSUM") as psxk,
                tc.tile_pool(name="gwork", bufs=1) as gwork,
                tc.tile_pool(name="psst", bufs=2, space="PSUM") as psst,
                tc.tile_pool(name="stt", bufs=2) as sttpool,
            ):
                gb3_sb = gwork.tile([1, 3 * U], F16)
                nc.sync.dma_start(out=gb3_sb[:], in_=gb3_d)
                gb1h_sb = gwork.tile([BL, U], F32)
                nc.sync.dma_start(out=gb1h_sb[:], in_=gb1h_d)

                pxk = [psxk.tile([BL, 512], F32, name=f"pxk{i}", tag=f"pxk{i}")
                       for i in range(6)]
                for k in range(KI):
                    for nk in range(6):
                        nc.tensor.matmul(pxk[nk][:], inpT[:, k, :],
                                         gk_tiles[k][:, ts(nk, 512)],
                                         start=(k == 0), stop=False,
                                         skip_group_check=True)
                for nk in range(6):
                    nc.tensor.matmul(pxk[nk][:], onesb_sb[0:1, 0:BL],
                                     gb3_sb[0:1, ts(nk, 512)],
                                     start=False, stop=True,
                                     skip_group_check=True)
                gk_cm.__exit__(None, None, None)

                z_sb = gwork.tile([BL, U], F32)
                r_sb = gwork.tile([BL, U], F32)
                for h in range(2):
                    nc.scalar.activation(z_sb[:, ts(h, 512)], pxk[h][:],
                                         mybir.ActivationFunctionType.Sigmoid)
                    nc.scalar.activation(r_sb[:, ts(h, 512)], pxk[2 + h][:],
                                         mybir.ActivationFunctionType.Sigmoid)
                t1 = gwork.tile([BL, U], F32)
                nc.vector.tensor_mul(t1[:], r_sb[:], gb1h_sb[:])
                hpre = gwork.tile([BL, U], F32)
                for h in range(2):
                    nc.vector.tensor_add(hpre[:, ts(h, 512)],
                                         t1[:, ts(h, 512)], pxk[4 + h][:])
                hh = gwork.tile([BL, U], F32)
                nc.scalar.activation(hh[:], hpre[:],
                                     mybir.ActivationFunctionType.Tanh)
                t3 = gwork.tile([BL, U], F32)
                nc.vector.tensor_mul(t3[:], z_sb[:], hh[:])
                state_sb = gwork.tile([BL, U], F32)
                nc.vector.tensor_sub(state_sb[:], hh[:], t3[:])
                nc.sync.dma_start(out=st_d, in_=state_sb[:])

                # state^T -> DRAM staging -> AllGather
                for j in range(KU):
                    pst = psst.tile([P, BL], F32)
                    nc.tensor.transpose(pst[:], state_sb[:, ts(j, P)],
                                        ident[0:BL, 0:BL])
                    stt = sttpool.tile([P, BL], F16)
                    nc.vector.tensor_copy(stt[:], pst[:])
                    nc.sync.dma_start(out=ag_in[ts(j, P), :], in_=stt[:])
                nc.gpsimd.collective_compute(
                    "AllGather", mybir.AluOpType.bypass,
                    replica_groups=[list(range(NCORES))],
                    ins=[ag_in.opt()], outs=[ag_out.opt()])

            # ---- fc: out = state_full @ fcw_shard + fcb -------------------
            with (
                tc.tile_pool(name="psfc", bufs=3, space="PSUM") as psfc,
                tc.tile_pool(name="osb", bufs=3) as opool,
                tc.tile_pool(name="fcb", bufs=1) as fcbpool,
            ):
                fcb_sb = fcbpool.tile([1, VL], F16)
                nc.sync.dma_start(out=fcb_sb[:], in_=fcb_d)
                stf = cpool.tile([P, KU, NCORES, BL], F16)
                ag_r = ag_out.rearrange("(c k p) b -> k p c b", c=NCORES, p=P)
                for k in range(KU):
                    nc.sync.dma_start(out=stf[:, k], in_=ag_r[k])
                for v in range(NVB):
                    for mc in range(2):
                        pfc = psfc.tile([P, VBLK], F32)
                        for kc in range(KU):
                            nc.tensor.matmul(
                                pfc[:],
                                stf[:, kc, ds(4 * mc, 4), :],
                                fw_tiles[v][:, kc, :],
                                start=(kc == 0), stop=False,
                                skip_group_check=True)
                        nc.tensor.matmul(pfc[:], onesb_sb[0:1, :],
                                         fcb_sb[0:1, ds(v * VBLK, VBLK)],
                                         start=False, stop=True,
                                         skip_group_check=True)
                        ou = opool.tile([P, VBLK], F32)
                        nc.scalar.copy(ou[:], pfc[:])
                        nc.sync.dma_start(out=out_d[ts(mc, P), ds(v * VBLK, VBLK)],
                                          in_=ou[:])
            fcw_cm.__exit__(None, None, None)

    nc.compile()
    return nc


def _host_prep(x, hidden, encode_output, embedding, W1, b1, W2, b2, V, bv,
               gru_kernel, gru_rec_kernel, gru_bias, fc_W, fc_b):
    f32, f16 = np.float32, np.float16
    x_idx = np.asarray(x).reshape(-1).astype(np.int64)
    xe = np.asarray(embedding, dtype=f32)[x_idx]          # (B, E)
    W1h = np.asarray(W1, f16)
    W2h = np.asarray(W2, f16)
    b12 = (np.asarray(b1, f32) + np.asarray(b2, f32)).reshape(1, U).astype(f16)
    v2 = np.zeros((U, 2), f16)
    v2[:, 0] = np.asarray(V, f32).reshape(-1).astype(f16)
    grukh = np.asarray(gru_kernel, f16)
    gb = np.asarray(gru_bias, f32)
    gb3 = np.concatenate([gb[0, :2 * U] + gb[1, :2 * U], gb[0, 2 * U:]])
    gb3 = gb3.reshape(1, 3 * U).astype(f16)
    gb1h = np.ascontiguousarray(
        np.broadcast_to(gb[1, 2 * U:].astype(f32), (BL, U)))
    expander = np.zeros((BL + 1, RSEQ), f16)
    for b in range(BL):
        expander[b, b * S:(b + 1) * S] = 1.0
    expander[BL, :] = 1.0
    ones1 = np.ones((1, P), f16)
    ones_bf = np.ones((1, P), f16)
    zeros_blk = np.zeros((P, 16 * BL), f16)
    fcwh = np.asarray(fc_W, f16)
    fcbh = np.asarray(fc_b, f32).reshape(1, V_SZ).astype(f16)
    ench = np.asarray(encode_output, f16)
    hidh = np.asarray(hidden, f16)
    xeh = np.asarray(xe, f16)

    in_maps = []
    for c in range(NCORES):
        sl = slice(c * BL, (c + 1) * BL)
        enc_c = np.ascontiguousarray(ench[sl].reshape(RSEQ, U))
        in_maps.append({
            "enc": enc_c,
            "encT": np.ascontiguousarray(enc_c.T),
            "hiddenT": np.ascontiguousarray(hidh[sl].T),
            "xeT": np.ascontiguousarray(xeh[sl].T),
            "w1": W1h, "w2": W2h, "b12": b12, "v2": v2,
            "gruk": grukh, "gbias3": gb3, "gb1h": gb1h,
            "expander": expander, "ones1": ones1, "ones_bf": ones_bf,
            "zeros_blk": zeros_blk,
            "fcw": np.ascontiguousarray(fcwh[:, c * VL:(c + 1) * VL]),
            "fcb": np.ascontiguousarray(fcbh[:, c * VL:(c + 1) * VL]),
        })
    return in_maps


def kernel(**inputs):
    global LAST_EXEC_NS
    if "nc" not in _CACHE:
        _CACHE["nc"] = _build()
    nc = _CACHE["nc"]
    in_maps = _host_prep(**inputs)
    res = run_bass_kernel_spmd(nc, in_maps, core_ids=list(range(NCORES)),
                               trace=PROFILE)
    LAST_EXEC_NS = res.exec_time_ns
    out = np.concatenate([res.results[c]["out"] for c in range(NCORES)], axis=1)
    state = np.concatenate([res.results[c]["state_out"] for c in range(NCORES)],
                           axis=0)
    attn_w = np.concatenate([res.results[c]["attnw_out"] for c in range(NCORES)],
                            axis=0)[:, :, None]
    return out.astype(np.float32), state.astype(np.float32), attn_w.astype(np.float32)
